# revision 10
# baseline (speedup 1.0000x reference)
"""AfmoeTokenChoiceRouter kernel for 8x Trainium2 NeuronCores.

Data-parallel over tokens: each of the 8 cores handles 2048 tokens (12.6 MB
of activations at 3 B/elem -> ~35 us HBM roofline per core at 358 GB/s).

Precision scheme (3 bytes per x element): x is split on the host into
fp16 hi (xh, 2 B) + e4m3 lo (xl8 = e4m3((x - xh) * 2^12), 1 B). The device
streams xh against the packed fp16 stationary [wh | wl] and xl8 against a
second fp16 stationary [w * 2^-12 | 0] (fp16 subnormals — the PE multiplies
them exactly; HW-probed), accumulating all terms in the same fp32 PSUM.
Result: ~2^-17-relative logits; top-8 selection matches a pure-fp32
reference on all but ~4 near-tie elements in 131072.

Per core pipeline (4 supertiles of 512 tokens, double/triple buffered):
  - DMA: xh/xl8 tiles in transposed [H, token] layout (host pre-packed to
    the exact SBUF layout, so every DMA is a contiguous burst)
  - PE: per k-chunk one [wh_c | wl_c] packed 128-wide stationary, matmul
    rhs=xh_c, then the scaled stationary with matmul rhs=xl8_c: psum rows
    0:64 accumulate the wh+corr terms, rows 64:128 the wl terms
  - PE: full [128,128] back-transposes -> [token, 2*64] layout
  - DVE adds the two 64-column halves (the hi/lo combine), ACT sigmoid
  - DVE top-8: max8/max_index on biased scores (exact fp32 selection),
    threshold mask + second max8 pass on masked unbiased scores, 8x8
    index-match to reorder into biased-rank order, normalize, scale by 2.5
Outputs per core: scores [128, 16, 8] f32 and indices [128, 16, 8] u32 in
partition-major token order (token = 128*tile + partition), unpermuted on
the host.
"""

import numpy as np

import concourse.bass as bass
import concourse.mybir as mybir
import concourse.tile as tile
import concourse.bass_utils as bass_utils
from concourse import bacc
from concourse.masks import make_identity

f32 = mybir.dt.float32
f16 = mybir.dt.float16
f8e4 = mybir.dt.float8e4
u32 = mybir.dt.uint32
Alu = mybir.AluOpType
Act = mybir.ActivationFunctionType

N_CORES = 8
T_FULL, H, E, TOPK = 16384, 2048, 64, 8
T_CORE = T_FULL // N_CORES          # 2048
TOK_ST = 512                        # tokens per supertile
N_ST = T_CORE // TOK_ST             # 4
TILES_ST = TOK_ST // 128            # 4
N_TILES = T_CORE // 128             # 16
N_CH = H // 128                     # 16 contraction chunks
ROUTE_SCALE = 2.5
XL_SCALE = 4096.0                   # 2^12: xl8 = e4m3(xl * 2^12), w2b = w * 2^-12


def router_body(tc, outs, ins, reps=1, skip_dma=False, skip_compute=False, n_terms=3, skip_topk=False):
    """Emit the per-core program. outs = (scores[128, N_TILES*8] f32,
    idx[128, N_TILES*8] u32); ins = (xh[N_ST,128,N_CH*TOK_ST] f16,
    xl[N_ST,128,N_CH*TOK_ST] f8e4, w2[128,N_CH*128] f16 (wh|wl packed),
    w2b same layout scaled 2^-12, bias[128,E] f32)."""
    nc = tc.nc
    out_s_d, out_i_d = outs
    xh_d, xl_d, w2_d, w2b_d, bias_d = ins

    with (
        tc.tile_pool(name="const", bufs=1) as constp,
        tc.tile_pool(name="xin", bufs=globals().get('_XBUFS', 3)) as xpool,
        tc.tile_pool(name="persist", bufs=1) as pers,
        tc.tile_pool(name="scratch", bufs=globals().get('_SCRBUFS', 3)) as scr,
        tc.tile_pool(name="ps_lt", bufs=globals().get('_LTBUFS', 3), space="PSUM") as ps_lt,
        tc.tile_pool(name="ps_l", bufs=globals().get('_PLBUFS', 3), space="PSUM") as ps_l,
    ):
        ident = constp.tile([128, 128], f32)
        make_identity(nc, ident[:])
        # setup DMAs ride the ACT HWDGE queue so they don't delay the first
        # x pieces on the sync queue (HWDGE is FIFO per issuing engine)
        w2_sb = constp.tile([128, N_CH, 128], f16)
        nc.scalar.dma_start(w2_sb[:], w2_d)
        w2b_sb = constp.tile([128, N_CH, 128], f16)
        nc.scalar.dma_start(w2b_sb[:], w2b_d)
        bias_sb = constp.tile([128, 1, E], f32)
        nc.scalar.dma_start(bias_sb[:], bias_d)

        # persistent per-core tensors
        s_all = pers.tile([128, N_TILES, E], f32)      # sigmoid scores
        b_all = pers.tile([128, N_TILES, E], f32)      # biased scores
        vb_all = pers.tile([128, N_TILES, 8], f32)     # top8 of biased
        vs_all = pers.tile([128, N_TILES, 8], f32)     # top8 of masked s
        ib_all = pers.tile([128, N_TILES, 8], u32)     # indices (biased order)
        is_all = pers.tile([128, N_TILES, 8], u32)     # indices (s order)
        ibf = pers.tile([128, N_TILES, 8], f32)
        isf = pers.tile([128, N_TILES, 8], f32)
        out_s_sb = pers.tile([128, N_TILES, 8], f32)

        DMA_CH = globals().get('_DMA_CH_OVERRIDE', 8)  # h-chunks per DMA piece

        def supertile(pos, tok_st):
            tiles_ss = tok_st // 128
            g, off = pos // TOK_ST, pos % TOK_ST
            t0 = pos // 128
            s4 = slice(t0, t0 + tiles_ss)
            xh_sb = xpool.tile([128, N_CH, tok_st], f16, tag="xh")
            xl_sb = xpool.tile([128, N_CH, tok_st], f8e4, tag="xl")
            tsl = slice(off, off + tok_st)
            xh_st = xh_d[g].rearrange("p (c t) -> p c t", t=TOK_ST)[:, :, tsl]
            xl_st = xl_d[g].rearrange("p (c t) -> p c t", t=TOK_ST)[:, :, tsl]
            if not skip_dma:
                xl_eng = nc.scalar if globals().get('_XL_ON_ACT', 1) else nc.sync
                pieces = globals().get('_PIECES', None)
                if pieces is None:
                    pieces = []
                    d0 = 0
                    while d0 < N_CH:
                        pieces.append((d0, min(DMA_CH, N_CH - d0)))
                        d0 += min(DMA_CH, N_CH - d0)
                for d0, dn in pieces:
                    dsl = slice(d0, d0 + dn)
                    nc.sync.dma_start(xh_sb[:, dsl, :], xh_st[:, dsl, :])
                    xl_eng.dma_start(xl_sb[:, dsl, :], xl_st[:, dsl, :])
            if skip_compute:
                return

            # GEMM: stationary [wh_c | wl_c] packed as one [128, 128] weight.
            # psum rows 0:64 accumulate wh terms, rows 64:128 wl terms; xh
            # streams against w2, xl8 against the 2^-12-scaled w2b (its
            # built-in scale cancels the host-side 2^12 on xl8 exactly).
            lt_ps = ps_lt.tile([128, tok_st], f32, tag="lt")
            for c in range(N_CH):
                w2_c = w2_sb[:, c, :]
                last = c == N_CH - 1
                nc.tensor.matmul(lt_ps[:], w2_c, xh_sb[:, c, :],
                                 start=(c == 0), stop=(last and n_terms == 1))
                if n_terms >= 3:
                    nc.tensor.matmul(lt_ps[:], w2b_sb[:, c, :], xl_sb[:, c, :],
                                     start=False, stop=last)
            if n_terms < 3:
                dummy = scr.tile([128, 1], f16, tag="dummy")
                nc.vector.tensor_copy(dummy[:], xl_sb[:, 0, :1])

            lt_sb = scr.tile([128, tok_st], f32, tag="ltsb")
            nc.scalar.copy(lt_sb[:], lt_ps[:])

            # full back-transpose per 128-token tile:
            # psum_l[:, q, 0:64] = wh-half logitsT.T, [:, q, 64:128] = wl-half
            l_ps = ps_l.tile([128, tiles_ss, 128], f32, tag="lps")
            for q in range(tiles_ss):
                nc.tensor.transpose(
                    l_ps[:, q, :],
                    lt_sb[:, q * 128:(q + 1) * 128],
                    ident[:],
                )

            # combine halves: logits[tok, e] = hi + lo (lanes aligned; DVE can
            # read only one PSUM operand, so stage the lo half through SBUF,
            # then add it back into the hi half in place -- sigmoid reads PSUM)
            half_sb = scr.tile([128, tiles_ss, E], f32, tag="half")
            nc.scalar.copy(half_sb[:], l_ps[:, :, E:2 * E])
            nc.vector.tensor_tensor(out=l_ps[:, :, 0:E], in0=l_ps[:, :, 0:E],
                                    in1=half_sb[:], op=Alu.add)

            s_sl = s_all[:, s4, :]
            nc.scalar.activation(s_sl, l_ps[:, :, 0:E], Act.Sigmoid)
            if skip_topk:
                nc.vector.tensor_copy(out_s_sb[:, s4, :], s_sl[:, :, :8])
                nc.vector.tensor_copy(ib_all[:, s4, :], s_sl[:, :, 8:16])
                return
            b_sl = b_all[:, s4, :]
            nc.vector.tensor_tensor(
                out=b_sl, in0=s_sl,
                in1=bias_sb[:].broadcast_to([128, tiles_ss, E]),
                op=Alu.add,
            )

            for q in range(tiles_ss):
                i = t0 + q
                nc.vector.max(out=vb_all[:, i, :], in_=b_all[:, i, :])
                nc.vector.max_index(out=ib_all[:, i, :], in_max=vb_all[:, i, :],
                                    in_values=b_all[:, i, :])

            # selected-expert masking: sarr = (b >= thr8) * s
            variant = globals().get('_TOPK_VARIANT', 0)
            sarr = scr.tile([128, tiles_ss, E], f32, tag="sarr")
            if variant in (1, 3):
                for q in range(tiles_ss):
                    i = t0 + q
                    nc.vector.scalar_tensor_tensor(
                        out=sarr[:, q, :], in0=b_all[:, i, :],
                        scalar=vb_all[:, i, 7:8], in1=s_all[:, i, :],
                        op0=Alu.is_ge, op1=Alu.mult)
            else:
                eng = nc.gpsimd if variant == 2 else nc.vector
                thr = vb_all[:, s4, 7:8].broadcast_to([128, tiles_ss, E])
                mask = scr.tile([128, tiles_ss, E], f32, tag="mask")
                eng.tensor_tensor(out=mask[:], in0=b_sl, in1=thr, op=Alu.is_ge)
                eng.tensor_tensor(out=sarr[:], in0=s_sl, in1=mask[:], op=Alu.mult)

            for q in range(tiles_ss):
                i = t0 + q
                nc.vector.max(out=vs_all[:, i, :], in_=sarr[:, q, :])
                nc.vector.max_index(out=is_all[:, i, :], in_max=vs_all[:, i, :],
                                    in_values=sarr[:, q, :])

            # reorder vs_all (s-descending) into biased-rank order by idx match
            nc.vector.tensor_copy(ibf[:, s4, :], ib_all[:, s4, :])
            nc.vector.tensor_copy(isf[:, s4, :], is_all[:, s4, :])
            eng2 = nc.gpsimd if variant in (2, 3) else nc.vector
            eq = scr.tile([128, tiles_ss, 8, 8], f32, tag="eq")
            eng2.tensor_tensor(
                out=eq[:],
                in0=ibf[:, s4, :].broadcast_to([128, tiles_ss, 8, 8]),
                in1=isf[:, s4, :][:, :, None, :].broadcast_to(
                    [128, tiles_ss, 8, 8]),
                op=Alu.is_equal,
            )
            g_sc = scr.tile([128, tiles_ss, 8, 8], f32, tag="g")
            eng2.tensor_tensor(
                out=g_sc[:], in0=eq[:],
                in1=vs_all[:, s4, :][:, :, None, :].broadcast_to(
                    [128, tiles_ss, 8, 8]),
                op=Alu.mult,
            )
            tsr = scr.tile([128, tiles_ss, 8], f32, tag="tsr")
            nc.vector.reduce_sum(out=tsr[:], in_=g_sc[:], axis=mybir.AxisListType.X)

            den = scr.tile([128, tiles_ss], f32, tag="den")
            nc.vector.reduce_sum(out=den[:], in_=vs_all[:, s4, :],
                                 axis=mybir.AxisListType.X)
            rec = scr.tile([128, tiles_ss], f32, tag="rec")
            nc.vector.reciprocal(rec[:], den[:])
            nc.vector.scalar_tensor_tensor(
                out=out_s_sb[:, s4, :], in0=tsr[:], scalar=ROUTE_SCALE,
                in1=rec[:].broadcast_to([128, tiles_ss, 8]),
                op0=Alu.mult, op1=Alu.mult,
            )
            od_s = out_s_d.rearrange("p (i k) -> p i k", k=8)
            od_i = out_i_d.rearrange("p (i k) -> p i k", k=8)
            nc.gpsimd.dma_start(od_s[:, s4, :], out_s_sb[:, s4, :])
            nc.gpsimd.dma_start(od_i[:, s4, :], ib_all[:, s4, :])

        # last supertiles shrink so the final serial DVE top-k tail is short
        schedule = globals().get('_SCHEDULE_OVERRIDE', [512, 512, 512, 384, 128])
        assert sum(schedule) == T_CORE

        def whole_pass():
            pos = 0
            for tok_st in schedule:
                supertile(pos, tok_st)
                pos += tok_st
            if skip_compute:
                return

        if reps == 1:
            whole_pass()
        else:
            with tc.For_i(0, reps, 1):
                whole_pass()


def build_nc(reps=1, skip_dma=False, skip_compute=False, n_terms=3, skip_topk=False):
    nc = bacc.Bacc("TRN2", target_bir_lowering=False, debug=False)
    xh_d = nc.dram_tensor("xh_d", [N_ST, 128, N_CH * TOK_ST], f16, kind="ExternalInput")
    xl_d = nc.dram_tensor("xl_d", [N_ST, 128, N_CH * TOK_ST], f8e4, kind="ExternalInput")
    w2_d = nc.dram_tensor("w2_d", [128, N_CH * 128], f16, kind="ExternalInput")
    w2b_d = nc.dram_tensor("w2b_d", [128, N_CH * 128], f16, kind="ExternalInput")
    bias_d = nc.dram_tensor("bias_d", [128, E], f32, kind="ExternalInput")
    out_s_d = nc.dram_tensor("out_s_d", [128, N_TILES * 8], f32, kind="ExternalOutput")
    out_i_d = nc.dram_tensor("out_i_d", [128, N_TILES * 8], u32, kind="ExternalOutput")

    with tile.TileContext(nc) as tc:
        router_body(
            tc,
            (out_s_d.ap(), out_i_d.ap()),
            (xh_d.ap(), xl_d.ap(), w2_d.ap(), w2b_d.ap(), bias_d.ap()),
            reps=reps, skip_dma=skip_dma, skip_compute=skip_compute,
            n_terms=n_terms, skip_topk=skip_topk,
        )
    nc.compile()
    return nc


def pack_x_shard(xh_shard_T):
    """[H, T_CORE] fp16 -> [N_ST, 128, N_CH*TOK_ST] in SBUF tile layout:
    out[st, p, c*TOK_ST + t] = xT[c*128 + p, st*TOK_ST + t]."""
    v = xh_shard_T.reshape(N_CH, 128, N_ST, TOK_ST)
    return np.ascontiguousarray(v.transpose(2, 1, 0, 3)).reshape(N_ST, 128, N_CH * TOK_ST)


def pack_w2(wh, wl):
    """wh/wl [E, H] fp16 -> [128, N_CH*128] with wh in cols 0:64, wl in 64:128
    of each chunk: out[p, c*128 + e] = (wh if e < E else wl)[e % E, c*128 + p]."""
    vh = wh.T.reshape(N_CH, 128, E)
    vl = wl.T.reshape(N_CH, 128, E)
    v = np.concatenate([vh, vl], axis=2)          # [N_CH, 128, 128]
    return np.ascontiguousarray(v.transpose(1, 0, 2)).reshape(128, N_CH * 128)


_NC_CACHE = {}


def prepare_in_maps(hidden_states, expert_bias, gate_w):
    x2 = np.asarray(hidden_states, dtype=np.float32).reshape(T_FULL, H)
    w = np.asarray(gate_w, dtype=np.float32)
    bias = np.asarray(expert_bias, dtype=np.float32)
    e4m3 = mybir.dt.np(f8e4)

    xh = x2.astype(np.float16)
    xl8 = ((x2 - xh.astype(np.float32)) * XL_SCALE).astype(e4m3)
    wh = w.astype(np.float16)
    wl = (w - wh.astype(np.float32)).astype(np.float16)
    whs = (w * (1.0 / XL_SCALE)).astype(np.float16)

    w2_p = pack_w2(wh, wl)
    w2b_p = pack_w2(whs, np.zeros_like(whs))
    bias_p = np.ascontiguousarray(np.broadcast_to(bias[None, :], (128, E)))

    in_maps = []
    for k in range(N_CORES):
        rows = slice(k * T_CORE, (k + 1) * T_CORE)
        in_maps.append({
            "xh_d": pack_x_shard(np.ascontiguousarray(xh[rows].T)),
            "xl_d": pack_x_shard(np.ascontiguousarray(xl8[rows].T)),
            "w2_d": w2_p,
            "w2b_d": w2b_p,
            "bias_d": bias_p,
        })
    return in_maps


def kernel(hidden_states, expert_bias, gate_w):
    in_maps = prepare_in_maps(hidden_states, expert_bias, gate_w)

    if "nc" not in _NC_CACHE:
        _NC_CACHE["nc"] = build_nc()
    nc = _NC_CACHE["nc"]

    res = bass_utils.run_bass_kernel_spmd(nc, in_maps, core_ids=list(range(N_CORES)))

    scores = np.empty((T_FULL, TOPK), dtype=np.float32)
    idx = np.empty((T_FULL, TOPK), dtype=np.int32)
    for k in range(N_CORES):
        o = res.results[k]
        s = o["out_s_d"].reshape(128, N_TILES, TOPK).transpose(1, 0, 2).reshape(T_CORE, TOPK)
        i = o["out_i_d"].view(np.int32).reshape(128, N_TILES, TOPK).transpose(1, 0, 2).reshape(T_CORE, TOPK)
        scores[k * T_CORE:(k + 1) * T_CORE] = s
        idx[k * T_CORE:(k + 1) * T_CORE] = i
    return scores, idx

